# revision 1
# baseline (speedup 1.0000x reference)
"""Distributed Trainium2 Bass kernel for nn_AttentionD_12412455485977.

3D-windowed multi-head attention with relative-position bias:
  qkv = x @ w_qkv ; per-head attention with bias gathered from rel_table
  via the static relative-position index; out = attn_out @ w_out + b_out.

Sharding: head-parallel. Core c computes head c for both batches and the
partial out-projection attn_out_h @ w_out[h]; the host sums the 8 partial
[2*2048, 128] results (the natural unshard of a head-sharded contraction).
b_out is added on core 0 only (scaled by the softmax denominator so the
in-kernel normalization leaves it intact).

Bias trick: bias[i, j] depends only on (pos_i - pos_j); with n ordered
z-major, the [2048, 2048] per-head bias matrix is block-Toeplitz over z
with 256x256 blocks indexed by dz = zi - zj.  exp factorizes across the
softmax: exp(s + b) = exp(s) * exp(b), so the kernel multiplies exp(scores)
by host-precomputed exp(bias) slabs that live entirely in SBUF.
"""

import os
import sys

import numpy as np

for _p in ("/opt/trn_rl_repo", "/root/.axon_site/_ro/trn_rl_repo"):
    if os.path.isdir(_p) and _p not in sys.path:
        sys.path.append(_p)

import ml_dtypes  # noqa: E402
import concourse.bass as bass  # noqa: E402
import concourse.tile as tile  # noqa: E402
from concourse import bacc, mybir  # noqa: E402
from concourse.bass_utils import run_bass_kernel_spmd  # noqa: E402

BF16 = mybir.dt.bfloat16
F32 = mybir.dt.float32
NPBF16 = ml_dtypes.bfloat16

B = 2            # batches
N = 2048         # tokens per batch (= 8*16*16, z-major)
C = 128          # channels
HEADS = 8
DH = 32          # head dim
D3, H3, W3 = 8, 16, 16
NCORES = 8

# ---------------------------------------------------------------------------
# host-side static index table for the exp(bias) slabs
# ---------------------------------------------------------------------------
# bias7[p, k*2048 + g*512 + ih] multiplies exp(scores^T) for the step with
# chunk/group offset k = ic - t + 3:
#   scores^T[j, i] tile with j = (4t+g)*128 + p, i = ic*512 + ih.
# biasT[j, i] = T[(zi-zj+7)*961 + (dy+15)*31 + (dx+15)]


def _bias7_index() -> np.ndarray:
    kk = np.arange(7)[:, None, None, None]
    gg = np.arange(4)[None, :, None, None]
    pp = np.arange(128)[None, None, :, None]
    ii = np.arange(512)[None, None, None, :]
    a = 2 * kk + 1 + ii // 256 - gg // 2          # zi - zj + 7
    pj = (gg % 2) * 128 + pp
    pi = ii % 256
    dy = pi // 16 - pj // 16 + 15
    dx = pi % 16 - pj % 16 + 15
    return (a * 961 + dy * 31 + dx).astype(np.int32)  # [7, 4, 128, 512]


_IDX7 = _bias7_index()

# ---------------------------------------------------------------------------
# device graph
# ---------------------------------------------------------------------------


def _build():
    nc = bacc.Bacc(None, target_bir_lowering=False, debug=False)

    xt_e = nc.declare_dram_parameter("xt", [C, B * N], BF16, isOutput=False)
    w3_e = nc.declare_dram_parameter("w3", [C, 96], BF16, isOutput=False)
    # [0:32, 0:128] = w_out head slice; [32, 0:128] = b_out (core 0 only);
    # col 128 = denominator passthrough (e_32) so the projection matmul also
    # transposes the softmax denominator into partition-major layout.
    waug_e = nc.declare_dram_parameter("waug", [DH + 1, C + 1], F32, isOutput=False)
    bias7_e = nc.declare_dram_parameter("bias7", [128, 7 * 2048], BF16, isOutput=False)
    out_e = nc.declare_dram_parameter("out", [B * N, C], F32, isOutput=True)

    with tile.TileContext(nc) as tc:
        with tc.tile_pool(name="persist", bufs=1) as persist:
            # DMA issue order tracks the critical path: phase-1 batch-0 needs
            # w3 + xt[0] immediately; everything else is deferred below.
            w3 = persist.tile([C, 96], BF16)
            nc.sync.dma_start(w3[:], w3_e[:])
            waug = persist.tile([DH + 1, C + 1], F32)
            nc.sync.dma_start(waug[:], waug_e[:])
            xt = [persist.tile([C, N], BF16, tag=f"xt{b}", name=f"xt{b}")
                  for b in range(B)]
            xt0_dma = nc.sync.dma_start(xt[0][:], xt_e[:, 0:N])
            bias7 = [persist.tile([128, 2048], BF16, tag=f"bias{k}",
                                  name=f"bias{k}") for k in range(7)]
            # chain the bulk loads behind the phase-1-critical xt[0] transfer
            # (slab k is first read at half-step k..; the serial chain keeps
            # each ~1.5us transfer off the critical DMA path but early enough)
            slab_dma = {}
            for k in range(7):
                slab_dma[k] = nc.gpsimd.dma_start(
                    bias7[k][:], bias7_e[:, k * 2048:(k + 1) * 2048])
            xt1_dma = nc.sync.dma_start(xt[1][:], xt_e[:, N:2 * N])
            # slab 3 (first used) may start once w3 is in; the rest are
            # gated on the per-batch shuffle DMAs below so the serialized
            # transfer path serves the latency-critical loads first.
            tile.add_dep_helper(slab_dma[3].ins, xt0_dma.ins, sync=True,
                                reason="slab3 after xt0")
            tile.add_dep_helper(xt1_dma.ins, slab_dma[3].ins, sync=True,
                                reason="xt1 after slab3")
            # touch the Exp table so the ~2.7us ACT_TABLE_LOAD overlaps
            # phase 1 instead of gating the first real exp.
            scratch = persist.tile([128, 1], F32)
            nc.vector.memset(scratch[:], 0.0)
            nc.scalar.activation(scratch[:], scratch[:],
                                 mybir.ActivationFunctionType.Exp)

            # per-batch tiles so phase 2 for batch 0 only depends on batch-0 prep
            qkT = [persist.tile([64, N], BF16, tag=f"qkT{b}", name=f"qkT{b}") for b in range(B)]
            qT4 = [persist.tile([128, N], BF16, tag=f"qT4{b}", name=f"qT4{b}") for b in range(B)]
            kT4 = [persist.tile([128, 512], BF16, tag=f"kT4{b}", name=f"kT4{b}") for b in range(B)]
            vaug = [persist.tile([128, 16 * 33], BF16, tag=f"vaug{b}", name=f"vaug{b}") for b in range(B)]

            # ---- phase 1: qkv projections -------------------------------
            with tc.tile_pool(name="ph1", bufs=2, space="PSUM") as ph1:
                for b in range(B):
                    nc.vector.memset(vaug[b][:], 1.0)
                    for ch in range(4):
                        qk_ps = ph1.tile([64, 512], F32)
                        nc.tensor.matmul(qk_ps[:], lhsT=w3[:, 0:64],
                                         rhs=xt[b][:, ch * 512:(ch + 1) * 512],
                                         start=True, stop=True)
                        nc.vector.tensor_copy(qkT[b][:, ch * 512:(ch + 1) * 512],
                                              qk_ps[:])
                    for tt in range(4):
                        v_ps = ph1.tile([128, 128], F32)
                        for u in range(4):
                            nt = tt * 4 + u
                            nc.tensor.matmul(v_ps[:, u * 32:(u + 1) * 32],
                                             lhsT=xt[b][:, nt * 128:(nt + 1) * 128],
                                             rhs=w3[:, 64:96],
                                             start=True, stop=True)
                        dst = vaug[b][:, tt * 132:(tt + 1) * 132]
                        dst = dst.rearrange("p (f c) -> p f c", f=4)[:, :, 0:DH]
                        src = v_ps[:].rearrange("p (f c) -> p f c", f=4)
                        nc.vector.tensor_copy(dst, src)
                    # replicate q across partition groups; scatter k by j-tile
                    for g in range(4):
                        shf = nc.sync.dma_start(qT4[b][32 * g:32 * (g + 1), :],
                                                qkT[b][0:32, :])
                        src = qkT[b][32:64, :].rearrange(
                            "d (t g jj) -> d t g jj", t=4, g=4, jj=128)[:, :, g, :]
                        dst = kT4[b][32 * g:32 * (g + 1), :].rearrange(
                            "d (t jj) -> d t jj", t=4)
                        shf = nc.sync.dma_start(dst, src)
                    # release the next tranche of bias-slab transfers
                    for k in ((2, 1, 0) if b == 0 else (4, 5, 6)):
                        tile.add_dep_helper(slab_dma[k].ins, shf.ins, sync=True,
                                            reason="slabs after shuffles")

            # ---- phase 2: attention ------------------------------------
            # half-steps of 2 j-tiles (2 PSUM banks) so the exp eviction of
            # one buffer overlaps the score matmuls filling the other.
            with (
                tc.tile_pool(name="score", bufs=2, space="PSUM") as score_pool,
                tc.tile_pool(name="outps", bufs=2, space="PSUM") as out_pool,
                tc.tile_pool(name="proj", bufs=1, space="PSUM") as proj_pool,
                tc.tile_pool(name="sb2", bufs=3) as sb2,
                tc.tile_pool(name="sb3", bufs=2) as sb3,
            ):
                def epilogue(b, ic, out_ps):
                    # evict PV psum, project (incl. denominator column and
                    # denominator-scaled b_out), normalize, store.
                    outT = sb3.tile([DH + 1, 512], F32, tag="outT", name="outT")
                    nc.vector.tensor_copy(outT[:], out_ps[:])
                    for half in range(2):
                        proj_ps = proj_pool.tile([128, 2 * (C + 1)], F32,
                                                 tag=f"proj{half}", bufs=1,
                                                 name=f"proj{half}")
                        for u in range(2):
                            it = 2 * half + u
                            nc.tensor.matmul(
                                proj_ps[:, u * (C + 1):(u + 1) * (C + 1)],
                                lhsT=outT[:, it * 128:(it + 1) * 128],
                                rhs=waug[:], start=True, stop=True)
                        pv = proj_ps[:].rearrange("p (f c) -> p f c", f=2)
                        recip = sb3.tile([128, 2], F32, tag=f"recip{half}",
                                         name=f"recip{half}")
                        nc.vector.reciprocal(recip[:],
                                             pv[:, :, C:C + 1].squeeze(2))
                        osb = sb3.tile([128, 2 * C], F32, tag=f"osb{half}",
                                       name=f"osb{half}")
                        ov = osb[:].rearrange("p (f c) -> p f c", f=2)
                        for u in range(2):
                            nc.vector.tensor_scalar_mul(
                                ov[:, u, :], pv[:, u, 0:C],
                                recip[:, u:u + 1])
                        for u in range(2):
                            it = 2 * half + u
                            row = b * N + ic * 512 + it * 128
                            nc.sync.dma_start(out_e[row:row + 128, :],
                                              ov[:, u, :])

                # software pipeline over global half-steps: emit step s's
                # score matmuls BEFORE step s-1's exp/mul/PV so the static
                # per-engine order keeps PE feeding ACT ahead of PV work.
                steps = []
                for b in range(B):
                    for ic in range(4):
                        for t in range(4):
                            for hh in range(2):
                                steps.append((b, ic, t, hh))
                out_ps_of = {}
                carry = None   # (emit_rest closure for step s-1)
                post = None    # epilogue closure for the chunk that ended
                for s, (b, ic, t, hh) in enumerate(steps):
                    if (t, hh) == (0, 0):
                        out_ps_of[(b, ic)] = out_pool.tile(
                            [DH + 1, 512], F32, name="out_ps", tag="out_ps")
                    score_ps = score_pool.tile([128, 1024], F32,
                                               name="score_ps", tag="score_ps")
                    for gg in range(2):
                        g = 2 * hh + gg
                        nc.tensor.matmul(
                            score_ps[:, gg * 512:(gg + 1) * 512],
                            lhsT=kT4[b][32 * g:32 * (g + 1),
                                        t * 128:(t + 1) * 128],
                            rhs=qT4[b][32 * g:32 * (g + 1),
                                       ic * 512:(ic + 1) * 512],
                            start=True, stop=True,
                            tile_position=(32 * g, 0))
                    if carry is not None:
                        carry()
                    if post is not None and (t, hh) >= (1, 0):
                        post()
                        post = None

                    def emit_rest(b=b, ic=ic, t=t, hh=hh, score_ps=score_ps):
                        expS = sb2.tile([128, 1024], BF16, tag="expS",
                                        name="expS")
                        nc.scalar.activation(expS[:], score_ps[:],
                                             mybir.ActivationFunctionType.Exp)
                        k7 = ic - t + 3
                        expT = sb2.tile([128, 1024], BF16, tag="expT",
                                        name="expT")
                        nc.vector.tensor_mul(
                            expT[:], expS[:],
                            bias7[k7][:, hh * 1024:(hh + 1) * 1024])
                        out_ps = out_ps_of[(b, ic)]
                        for gg in range(2):
                            jt = 4 * t + 2 * hh + gg
                            nc.tensor.matmul(
                                out_ps[:],
                                lhsT=vaug[b][:, jt * 33: jt * 33 + 33],
                                rhs=expT[:, gg * 512:(gg + 1) * 512],
                                start=(t == 0 and hh == 0 and gg == 0),
                                stop=(t == 3 and hh == 1 and gg == 1),
                                skip_group_check=True)

                    carry = emit_rest
                    if (t, hh) == (3, 1):
                        post = (lambda b=b, ic=ic:
                                epilogue(b, ic, out_ps_of[(b, ic)]))
                carry()
                post()

    nc.compile()
    return nc


_NC = None


def _get_nc():
    global _NC
    if _NC is None:
        _NC = _build()
    return _NC


# ---------------------------------------------------------------------------
# host side
# ---------------------------------------------------------------------------


def _prep_in_maps(x, w_qkv, rel_table, w_out, b_out):
    x = np.asarray(x, np.float32)
    w_qkv = np.asarray(w_qkv, np.float32)
    rel_table = np.asarray(rel_table, np.float32)
    w_out = np.asarray(w_out, np.float32)
    b_out = np.asarray(b_out, np.float32)

    scale = DH ** -0.5
    xt = np.ascontiguousarray(x.transpose(2, 0, 1).reshape(C, B * N)).astype(NPBF16)

    in_maps = []
    for hc in range(NCORES):
        w3 = np.concatenate([
            w_qkv[:, hc * DH:(hc + 1) * DH] * scale,
            w_qkv[:, 256 + hc * DH: 256 + (hc + 1) * DH],
            w_qkv[:, 512 + hc * DH: 512 + (hc + 1) * DH],
        ], axis=1).astype(NPBF16)
        waug = np.zeros((DH + 1, C + 1), np.float32)
        waug[0:DH, 0:C] = w_out[hc * DH:(hc + 1) * DH, :]
        if hc == 0:
            waug[DH, 0:C] = b_out
        waug[DH, C] = 1.0
        bias7 = np.exp(rel_table[:, hc][_IDX7])            # [7, 4, 128, 512]
        bias7 = np.ascontiguousarray(
            bias7.transpose(2, 0, 1, 3).reshape(128, 7 * 2048)).astype(NPBF16)
        in_maps.append({
            "xt": xt,
            "w3": np.ascontiguousarray(w3),
            "waug": waug,
            "bias7": bias7,
        })
    return in_maps


def _run(in_maps, **kwargs):
    nc = _get_nc()
    return run_bass_kernel_spmd(nc, in_maps, core_ids=list(range(NCORES)), **kwargs)


def kernel(x, w_qkv, rel_table, w_out, b_out, d=None, h=None, w=None):
    in_maps = _prep_in_maps(x, w_qkv, rel_table, w_out, b_out)
    res = _run(in_maps)
    acc = np.zeros((B * N, C), np.float64)
    for i in range(NCORES):
        acc += res.results[i]["out"].astype(np.float64)
    return acc.reshape(B, N, C).astype(np.float32)

